# revision 1
# baseline (speedup 1.0000x reference)
"""Trainium2 Bass kernel for nn_Attention_40510131535961.

The reference module applies softmax over a size-1 axis, so the attention
weights are identically 1.0 and the whole attn MLP (W1/b1/W2/b2, LeakyReLU)
is dead code.  The output reduces to

    context[b, 0, e] = sum_s encode_output[b, s, e]        # [32, 1, 1024]

Strategy: data-parallel over batch across 8 NeuronCores (4 batches/core).
Per core, stream the [4, 2048, 1024] f32 shard through SBUF in 2 MiB DMAs
([128 s-partitions, 4 s-subchunks, 1024 e] tiles), fold each chunk to
[128, 1024] on VectorE as it lands and merge into a per-batch accumulator,
then reduce the partition axis with a ones-vector matmul on TensorE into
PSUM.  The kernel is HBM-bound: ~32 MiB/core at the ~358 GB/s per-core HBM
share (~90 us floor); measured ~105 us including fixed preamble/epilogue.
"""

import sys
import types

import numpy as np

import concourse.bacc as bacc
import concourse.bass as bass
import concourse.mybir as mybir
import concourse.tile as tile
from concourse.bass_utils import run_bass_kernel_spmd


def _ensure_ntff_hook():
    """bass_utils imports antenv.axon_hooks when tracing is requested (e.g.
    BASS_TRACE=1 in the environment); this image's antenv lacks that module,
    which would hard-crash instead of degrading.  Synthesize it from the
    trn_agent_boot ctypes shim, best-effort."""
    try:
        import antenv.axon_hooks  # noqa: F401
        return
    except ImportError:
        pass
    try:
        import antenv
        from trn_agent_boot.trn_boot import _ntff_profile_via_ctypes

        hook = _ntff_profile_via_ctypes("/opt/axon/libaxon_pjrt.so")
        mod = types.ModuleType("antenv.axon_hooks")
        mod.get_axon_ntff_profile_hook = lambda: hook
        mod.set_axon_ntff_profile_hook = lambda h: None
        sys.modules["antenv.axon_hooks"] = mod
        antenv.axon_hooks = mod
    except Exception:
        pass

N_CORES = 8
B, S, E = 32, 2048, 1024
BP = B // N_CORES      # batches per core
P = 128                # SBUF partitions
F32 = mybir.dt.float32

_CACHE = {}


def _build_nc() -> bass.Bass:
    # Bacc (not raw Bass): its compile()/finalize() runs
    # generate_event_semaphores(), which splits multi-sem waits into
    # InstEventSemaphore — TRN2 instructions support at most 1 wait.
    nc = bacc.Bacc()
    x = nc.declare_dram_parameter("x", [BP, S, E], F32, isOutput=False)
    y = nc.declare_dram_parameter("y", [BP, E], F32, isOutput=True)

    # s = n*P + p  ->  16 s-subchunks of [P, E] per batch
    xr = x[:].rearrange("b (n p) e -> b n p e", p=P)

    # Uniform 2 MiB DMAs keep the HBM stream at ~400 GB/s (mixed sizes and
    # PE-side accumulation both measured slower).  Each chunk is folded to
    # width E on VectorE as soon as it lands and merged into a per-batch
    # accumulator (the first chunk's prefix, in place).  The LAST batch
    # tapers its final chunks so the serial DVE tail after the last DMA
    # byte is short.  The cross-partition reduce is a ones-matmul into PSUM.
    PATTERNS = [[4, 4, 4, 4]] * (BP - 1) + [[4, 4, 4, 2, 1, 1]]
    NH = E // 512

    with tile.TileContext(nc) as tc:
        with (
            tc.tile_pool(name="inp4", bufs=8) as pin4,
            tc.tile_pool(name="inp2", bufs=1) as pin2,
            tc.tile_pool(name="inp1", bufs=2) as pin1,
            tc.tile_pool(name="small", bufs=1) as psm,
            tc.tile_pool(name="ps", bufs=4, space="PSUM") as pps,
        ):
            pool_by_sz = {4: pin4, 2: pin2, 1: pin1}
            ones = psm.tile([P, 1], F32)
            nc.vector.memset(ones[:], 1.0)
            out_sb = psm.tile([1, BP * E], F32)

            for b in range(BP):
                pattern = PATTERNS[b]
                acc = None
                off = 0
                for ci, sz in enumerate(pattern):
                    t = pool_by_sz[sz].tile([P, sz, E], F32, tag=f"c{sz}")
                    nc.sync.dma_start(
                        t[:], xr[b, off : off + sz].rearrange("n p e -> p n e")
                    )
                    off += sz
                    flat = t[:].rearrange("p k e -> p (k e)")
                    # fold chunk to width E in place (sz is a power of two)
                    w = sz * E
                    while w > E:
                        w //= 2
                        nc.vector.tensor_add(
                            flat[:, :w], flat[:, :w], flat[:, w : 2 * w]
                        )
                    if acc is None:
                        acc = flat  # [128, :E] prefix of the first chunk
                    else:
                        nc.vector.tensor_add(acc[:, :E], acc[:, :E], flat[:, :E])
                # partition-axis reduce via ones-matmul (fp32, N<=512/bank)
                for h in range(NH):
                    ps = pps.tile([1, 512], F32, tag="ps", name=f"ps_{b}_{h}")
                    nc.tensor.matmul(ps[:], ones[:], acc[:, h * 512 : (h + 1) * 512])
                    # PSUM->SBUF copies on ACT only: a DVE copy here would
                    # make the in-order DVE queue wait for PE's matmul
                    # before starting the next batch's folds
                    nc.scalar.copy(
                        out_sb[:, b * E + h * 512 : b * E + (h + 1) * 512], ps[:]
                    )
                # per-batch 4 KiB output DMA on the ACT HWDGE ring: SP's
                # queue is FIFO, so nc.sync here would block later input-DMA
                # issues behind this batch's reduction chain.  (Keep APs 2D:
                # 1D DRAM APs break NEFF load on this stack.)
                nc.scalar.dma_start(y[b : b + 1, :], out_sb[:1, b * E : (b + 1) * E])
    return nc


def _get_nc() -> bass.Bass:
    if "nc" not in _CACHE:
        nc = _build_nc()
        nc.finalize()
        _CACHE["nc"] = nc
    return _CACHE["nc"]


def _run(encode_output: np.ndarray, **spmd_kwargs):
    _ensure_ntff_hook()
    enc = np.ascontiguousarray(np.asarray(encode_output, dtype=np.float32))
    assert enc.shape == (B, S, E), enc.shape
    in_maps = [{"x": enc[i * BP : (i + 1) * BP]} for i in range(N_CORES)]
    res = run_bass_kernel_spmd(_get_nc(), in_maps, list(range(N_CORES)), **spmd_kwargs)
    out = np.concatenate([res.results[i]["y"] for i in range(N_CORES)], axis=0)
    return out.reshape(B, 1, E), res


def kernel(encode_output, hidden_state=None, W1=None, b1=None, W2=None, b2=None):
    out, _ = _run(encode_output)
    return out



# revision 3
# speedup vs baseline: 1.0294x; 1.0294x over previous
"""Trainium2 Bass kernel for nn_Attention_40510131535961.

The reference module applies softmax over a size-1 axis, so the attention
weights are identically 1.0 and the whole attn MLP (W1/b1/W2/b2, LeakyReLU)
is dead code.  The output reduces to

    context[b, 0, e] = sum_s encode_output[b, s, e]        # [32, 1, 1024]

Strategy: data-parallel over batch across 8 NeuronCores (4 batches/core).
Per core, stream the [4, 2048, 1024] f32 shard through SBUF in 2 MiB DMAs
([128 s-partitions, 4 s-subchunks, 1024 e] tiles), fold each chunk to
[128, 1024] on VectorE as it lands and merge into a per-batch accumulator,
then reduce the partition axis with a ones-vector matmul on TensorE into
PSUM.  The kernel is HBM-bound: ~32 MiB/core at the ~358 GB/s per-core HBM
share (~90 us floor); measured ~105 us including fixed preamble/epilogue.
"""

import sys
import types

import numpy as np

import concourse.bacc as bacc
import concourse.bass as bass
import concourse.mybir as mybir
import concourse.tile as tile
from concourse.bass_utils import run_bass_kernel_spmd


def _ensure_ntff_hook():
    """bass_utils imports antenv.axon_hooks when tracing is requested (e.g.
    BASS_TRACE=1 in the environment); this image's antenv lacks that module,
    which would hard-crash instead of degrading.  Synthesize it from the
    trn_agent_boot ctypes shim, best-effort."""
    try:
        import antenv.axon_hooks  # noqa: F401
        return
    except ImportError:
        pass
    try:
        import antenv
        from trn_agent_boot.trn_boot import _ntff_profile_via_ctypes

        hook = _ntff_profile_via_ctypes("/opt/axon/libaxon_pjrt.so")
        mod = types.ModuleType("antenv.axon_hooks")
        mod.get_axon_ntff_profile_hook = lambda: hook
        mod.set_axon_ntff_profile_hook = lambda h: None
        sys.modules["antenv.axon_hooks"] = mod
        antenv.axon_hooks = mod
    except Exception:
        pass

N_CORES = 8
B, S, E = 32, 2048, 1024
BP = B // N_CORES      # batches per core
P = 128                # SBUF partitions
F32 = mybir.dt.float32

_CACHE = {}


def _build_nc() -> bass.Bass:
    # Bacc (not raw Bass): its compile()/finalize() runs
    # generate_event_semaphores(), which splits multi-sem waits into
    # InstEventSemaphore — TRN2 instructions support at most 1 wait.
    nc = bacc.Bacc()
    x = nc.declare_dram_parameter("x", [BP, S, E], F32, isOutput=False)
    y = nc.declare_dram_parameter("y", [BP, E], F32, isOutput=True)

    xf = x[:]

    # Uniform 2 MiB DMAs keep the HBM stream at ~400 GB/s (mixed sizes and
    # PE-side accumulation both measured slower).  Each chunk is folded to
    # width E on VectorE as soon as it lands and merged into a per-batch
    # accumulator (the first chunk's prefix, in place).  The LAST batch
    # tapers its final chunks so the serial DVE tail after the last DMA
    # byte is short.  The cross-partition reduce is a ones-matmul into PSUM.
    PATTERNS = [[4, 4, 4, 4]] * (BP - 1) + [[4, 4, 4, 2, 1, 1]]
    NH = E // 512

    with tile.TileContext(nc) as tc:
        with (
            tc.tile_pool(name="inp4", bufs=8) as pin4,
            tc.tile_pool(name="inp2", bufs=1) as pin2,
            tc.tile_pool(name="inp1", bufs=2) as pin1,
            tc.tile_pool(name="small", bufs=1) as psm,
            tc.tile_pool(name="ps", bufs=4, space="PSUM") as pps,
        ):
            pool_by_sz = {4: pin4, 2: pin2, 1: pin1}
            ones = psm.tile([P, 1], F32)
            nc.vector.memset(ones[:], 1.0)
            out_sb = psm.tile([1, BP * E], F32)

            for b in range(BP):
                pattern = PATTERNS[b]
                acc = None
                off = 0
                for ci, sz in enumerate(pattern):
                    t = pool_by_sz[sz].tile([P, sz, E], F32, tag=f"c{sz}")
                    flat = t[:].rearrange("p k e -> p (k e)")
                    # contiguous sz*4KiB HBM run per partition row: row p
                    # covers s in [off*P + p*sz, off*P + (p+1)*sz)
                    nc.sync.dma_start(
                        flat,
                        xf[b, off * P : (off + sz) * P].rearrange(
                            "(p m) e -> p (m e)", p=P
                        ),
                    )
                    off += sz
                    # fold chunk to width E in place (sz is a power of two)
                    w = sz * E
                    while w > E:
                        w //= 2
                        nc.vector.tensor_add(
                            flat[:, :w], flat[:, :w], flat[:, w : 2 * w]
                        )
                    if acc is None:
                        acc = flat  # [128, :E] prefix of the first chunk
                    else:
                        nc.vector.tensor_add(acc[:, :E], acc[:, :E], flat[:, :E])
                # partition-axis reduce via ones-matmul (fp32, N<=512/bank)
                for h in range(NH):
                    ps = pps.tile([1, 512], F32, tag="ps", name=f"ps_{b}_{h}")
                    nc.tensor.matmul(ps[:], ones[:], acc[:, h * 512 : (h + 1) * 512])
                    # PSUM->SBUF copies on ACT only: a DVE copy here would
                    # make the in-order DVE queue wait for PE's matmul
                    # before starting the next batch's folds
                    nc.scalar.copy(
                        out_sb[:, b * E + h * 512 : b * E + (h + 1) * 512], ps[:]
                    )
                # per-batch 4 KiB output DMA on the ACT HWDGE ring: SP's
                # queue is FIFO, so nc.sync here would block later input-DMA
                # issues behind this batch's reduction chain.  (Keep APs 2D:
                # 1D DRAM APs break NEFF load on this stack.)
                nc.scalar.dma_start(y[b : b + 1, :], out_sb[:1, b * E : (b + 1) * E])
    return nc


def _get_nc() -> bass.Bass:
    if "nc" not in _CACHE:
        nc = _build_nc()
        nc.finalize()
        _CACHE["nc"] = nc
    return _CACHE["nc"]


def _run(encode_output: np.ndarray, **spmd_kwargs):
    _ensure_ntff_hook()
    enc = np.ascontiguousarray(np.asarray(encode_output, dtype=np.float32))
    assert enc.shape == (B, S, E), enc.shape
    in_maps = [{"x": enc[i * BP : (i + 1) * BP]} for i in range(N_CORES)]
    res = run_bass_kernel_spmd(_get_nc(), in_maps, list(range(N_CORES)), **spmd_kwargs)
    out = np.concatenate([res.results[i]["y"] for i in range(N_CORES)], axis=0)
    return out.reshape(B, 1, E), res


def kernel(encode_output, hidden_state=None, W1=None, b1=None, W2=None, b2=None):
    out, _ = _run(encode_output)
    return out



# revision 11
# speedup vs baseline: 1.0870x; 1.0560x over previous
"""Trainium2 Bass kernel for nn_Attention_40510131535961.

The reference module applies softmax over a size-1 axis, so the attention
weights are identically 1.0 and the whole attn MLP (W1/b1/W2/b2, LeakyReLU)
is dead code.  The output reduces to

    context[b, 0, e] = sum_s encode_output[b, s, e]        # [32, 1, 1024]

Strategy: data-parallel over batch across 8 NeuronCores (4 batches/core).
Per core, stream the [4, 2048, 1024] f32 shard through SBUF in 2 MiB DMAs
with CONTIGUOUS 16 KiB HBM runs per partition row (row p of a chunk covers
s in [off*P + p*sz, off*P + (p+1)*sz)); the 16 per-core DMA engines are
byte-rate-bound (~26 GB/s each, ~410-435 GB/s/core aggregate), so the
~82 us stream phase is the hard floor.  As each chunk lands, DVE folds it
in place to [128, E] (log2 width-halving adds); PE then accumulates the
folded chunk straight into per-batch PSUM banks with single-pass float32r
ones-matmuls (start/stop over the batch's chunks) — no DVE merge pass, so
DVE tracks the stream with slack and the tail after the last input byte is
just: PE matmul on the (tapered, fold-free) final subchunk -> PSUM->SBUF
copy -> 4 KiB output DMA.  Copies for early batches ride ACT so the DVE
queue never waits on PE; the last batch's copies ride the then-idle DVE.
"""

import sys
import types

import numpy as np

import concourse.bacc as bacc
import concourse.bass as bass
import concourse.mybir as mybir
import concourse.tile as tile
from concourse.bass_utils import run_bass_kernel_spmd


def _ensure_ntff_hook():
    """bass_utils imports antenv.axon_hooks when tracing is requested (e.g.
    BASS_TRACE=1 in the environment); this image's antenv lacks that module,
    which would hard-crash instead of degrading.  Synthesize it from the
    trn_agent_boot ctypes shim, best-effort."""
    try:
        import antenv.axon_hooks  # noqa: F401
        return
    except ImportError:
        pass
    try:
        import antenv
        from trn_agent_boot.trn_boot import _ntff_profile_via_ctypes

        hook = _ntff_profile_via_ctypes("/opt/axon/libaxon_pjrt.so")
        mod = types.ModuleType("antenv.axon_hooks")
        mod.get_axon_ntff_profile_hook = lambda: hook
        mod.set_axon_ntff_profile_hook = lambda h: None
        sys.modules["antenv.axon_hooks"] = mod
        antenv.axon_hooks = mod
    except Exception:
        pass

N_CORES = 8
B, S, E = 32, 2048, 1024
BP = B // N_CORES      # batches per core
P = 128                # SBUF partitions
F32 = mybir.dt.float32
F32R = mybir.dt.float32r

_CACHE = {}


def _build_nc() -> bass.Bass:
    # Bacc (not raw Bass): its compile()/finalize() runs
    # generate_event_semaphores(), which splits multi-sem waits into
    # InstEventSemaphore — TRN2 instructions support at most 1 wait.
    nc = bacc.Bacc()
    x = nc.declare_dram_parameter("x", [BP, S, E], F32, isOutput=False)
    y = nc.declare_dram_parameter("y", [BP, E], F32, isOutput=True)
    xf = x[:]

    # Chunk patterns in units of [P, E] subchunks.  The LAST batch tapers
    # so the final chunks need no DVE fold (sz=1 goes straight to PE) and
    # the serial tail after the last DMA byte is short.
    PATTERNS = [[4, 4, 4, 4]] * (BP - 1) + [[4, 4, 4, 2, 2]]

    with tile.TileContext(nc) as tc:
        with (
            tc.tile_pool(name="inp4", bufs=7) as pin4,
            tc.tile_pool(name="inp2", bufs=2) as pin2,
            tc.tile_pool(name="red", bufs=8) as pred,
            tc.tile_pool(name="small", bufs=1) as psm,
            tc.tile_pool(name="ps", bufs=8, space="PSUM") as pps,
        ):
            pool_by_sz = {4: pin4, 2: pin2}
            # memset can't write float32r directly (codegen rejects the set
            # value type), so memset f32 staging and round-copy on DVE
            ones_f = psm.tile([P, 1], F32)
            nc.vector.memset(ones_f[:], 1.0)
            ones = psm.tile([P, 1], F32R)
            nc.vector.tensor_copy(ones[:], ones_f[:])
            ones_r = ones[:]
            out_sb = psm.tile([1, BP * E], F32)

            for b in range(BP):
                pattern = PATTERNS[b]
                last_ci = len(pattern) - 1
                psA = pps.tile([1, 512], F32, tag="ps", name=f"psA_{b}")
                psB = pps.tile([1, 512], F32, tag="ps", name=f"psB_{b}")
                off = 0
                for ci, sz in enumerate(pattern):
                    t = pool_by_sz[sz].tile([P, sz, E], F32, tag=f"c{sz}")
                    flat = t[:].rearrange("p k e -> p (k e)")
                    # contiguous sz*4KiB HBM run per partition row
                    nc.sync.dma_start(
                        flat,
                        xf[b, off * P : (off + sz) * P].rearrange(
                            "(p m) e -> p (m e)", p=P
                        ),
                    )
                    off += sz
                    # fold chunk to width E (sz is a power of two >= 2);
                    # intermediate adds run in place, the final add writes a
                    # dedicated float32r tile — the BIR verifier requires
                    # every writer of an FP32r matmul input to round to
                    # FP32r, so the raw DMA tile can't feed PE directly.
                    # (Bonus: the input buffer is free for DMA reuse as soon
                    # as the fold is done, without waiting on PE.)
                    red = pred.tile([P, E], F32R, tag="red")
                    w = sz * E
                    while w > 2 * E:
                        w //= 2
                        nc.vector.tensor_add(
                            flat[:, :w], flat[:, :w], flat[:, w : 2 * w]
                        )
                    nc.vector.tensor_add(red[:], flat[:, :E], flat[:, E : 2 * E])
                    # accumulate the folded [P, E] into this batch's PSUM
                    # banks: single-pass float32r ones-matmul (precision is
                    # ample for the 2e-2 gate; data folds stayed exact f32)
                    st = ci == 0
                    sp = ci == last_ci
                    nc.tensor.matmul(
                        psA[:], ones_r, red[:, 0:512], start=st, stop=sp,
                    )
                    nc.tensor.matmul(
                        psB[:], ones_r, red[:, 512:1024], start=st, stop=sp,
                    )
                # PSUM->SBUF: ACT for early batches (keeps the in-order DVE
                # queue from waiting on PE mid-stream); DVE for the last
                # batch, where DVE is already drained and ACT's ~0.7us/copy
                # would sit on the critical tail.
                if b == BP - 1:
                    nc.vector.tensor_copy(out_sb[:, b * E : b * E + 512], psA[:])
                    nc.vector.tensor_copy(out_sb[:, b * E + 512 : (b + 1) * E], psB[:])
                else:
                    nc.scalar.copy(out_sb[:, b * E : b * E + 512], psA[:])
                    nc.scalar.copy(out_sb[:, b * E + 512 : (b + 1) * E], psB[:])
                # per-batch 4 KiB output DMA on the ACT HWDGE ring: SP's
                # queue is FIFO, so nc.sync here would block later input-DMA
                # issues behind this batch's reduction chain.  (Keep APs 2D:
                # 1D DRAM APs break NEFF load on this stack.)
                nc.scalar.dma_start(y[b : b + 1, :], out_sb[:1, b * E : (b + 1) * E])
    return nc


def _get_nc() -> bass.Bass:
    if "nc" not in _CACHE:
        nc = _build_nc()
        nc.finalize()
        _CACHE["nc"] = nc
    return _CACHE["nc"]


def _run(encode_output: np.ndarray, **spmd_kwargs):
    _ensure_ntff_hook()
    enc = np.ascontiguousarray(np.asarray(encode_output, dtype=np.float32))
    assert enc.shape == (B, S, E), enc.shape
    in_maps = [{"x": enc[i * BP : (i + 1) * BP]} for i in range(N_CORES)]
    res = run_bass_kernel_spmd(_get_nc(), in_maps, list(range(N_CORES)), **spmd_kwargs)
    out = np.concatenate([res.results[i]["y"] for i in range(N_CORES)], axis=0)
    return out.reshape(B, 1, E), res


def kernel(encode_output, hidden_state=None, W1=None, b1=None, W2=None, b2=None):
    out, _ = _run(encode_output)
    return out
